# revision 50
# baseline (speedup 1.0000x reference)
"""AlphaFold-style gated MSA attention on 8 Trainium2 NeuronCores.

Batch-sharded (128 batches -> 16 per core). Full inputs in, full output out.

Math per batch b (reference):
  q = (q_data @ Wq) * hk^-0.5          [Q, H, 32]
  k = m_data @ Wk ; v = m_data @ Wv    [K, H, 32]
  S[h] = q_h k_h^T + bias[b] + nb[h]   [H, Q, K]
  w = softmax(S, axis=-1)
  wa = w @ v                            [Q, H, 32]
  gate = sigmoid(q_data @ Wg + gb)
  out = (wa * gate).reshape(Q, 256) @ Wo + o_bias

Device-side formulation (per core, layouts chosen so no transposes are
needed on-device):
  S^T[k, q] computed head-by-head from k^T/q^T projections (feature dim on
  partitions).  softmax is done unnormalized with the bias adds replaced by
  multiplies of host-precomputed exp(bias)^T ("eb") and exp(nb)^T ("en"):
      w^T = exp(S^T) * en_h * eb          (bf16)
  The V-matmul uses lhsT = [v_h | 2.0] so PSUM row 32 accumulates 2*sum_k w,
  giving the softmax denominators for free.  Normalization and gating fuse:
      ga^T = wa^T * (1 + tanh(x/2 + gb/2)) * recip(2*sum) = wa^T*sigmoid/sum
  with the per-head recip broadcast across 32 partitions by a tiny indicator
  matmul.  Output projection back to [q, 256] with o_bias added during PSUM
  evacuation.
"""

import os
import sys

sys.path.insert(0, "/opt/trn_rl_repo")

import numpy as np
import ml_dtypes
from contextlib import ExitStack

import concourse.bass as bass  # noqa: F401  (engine types)
import concourse.bacc as bacc
import concourse.mybir as mybir
import concourse.tile as tile

BF16 = ml_dtypes.bfloat16

NUM_CORES = 8
B, Q, K, A = 128, 384, 384, 256
H, HD = 8, 32  # heads, head dim
OUT = 256
BPC = B // NUM_CORES  # batches per core


PAIR_MUL = __import__("os").environ.get("PAIR_MUL", "0") == "1"
GPS_HEADS = tuple(int(x) for x in __import__("os").environ.get("GPS_HEADS", "9").split(","))


def _build_body(ctx, tc, io, bpc):
    nc = tc.nc
    f32, bf = mybir.dt.float32, mybir.dt.bfloat16
    Exp = mybir.ActivationFunctionType.Exp
    Tanh = mybir.ActivationFunctionType.Tanh
    MUL, ADD = mybir.AluOpType.mult, mybir.AluOpType.add

    const = ctx.enter_context(tc.tile_pool(name="const", bufs=1))
    lp = ctx.enter_context(tc.tile_pool(name="loads", bufs=int(__import__("os").environ.get("LP_BUFS", "5"))))
    pp = ctx.enter_context(tc.tile_pool(name="proj", bufs=int(__import__("os").environ.get("PP_BUFS", "2"))))
    wp = ctx.enter_context(tc.tile_pool(name="work", bufs=int(__import__("os").environ.get("WP_BUFS", "4"))))
    wap = ctx.enter_context(tc.tile_pool(name="wa", bufs=8))
    gp = ctx.enter_context(tc.tile_pool(name="gating", bufs=int(__import__("os").environ.get("GP_BUFS", "2"))))
    outp = ctx.enter_context(tc.tile_pool(name="outp", bufs=3))
    # PSUM: 2 x 3 banks (S^T) + 2 x 1 bank (everything else) = 8 banks.
    Sp = ctx.enter_context(tc.tile_pool(name="psum_S", bufs=2, space="PSUM"))
    sp = ctx.enter_context(tc.tile_pool(name="psum_sm", bufs=2, space="PSUM"))

    # ---- resident constants ----
    en_sb = const.tile([128, H, 3, Q], bf, tag="en")
    nc.sync.dma_start(en_sb[:], io["enT"])
    w_sb = {}
    for name in ("wq", "wk", "wv", "wg", "wo"):
        w_sb[name] = const.tile([128, 2, 256], bf, tag=name, name=name)
        nc.sync.dma_start(w_sb[name][:], io[name])
    # o_bias as a [1, 256] row plus a [1, 128] ones row for the rank-1
    # PSUM-accumulate trick (bf16 to match the other matmul operands)
    obias_row = const.tile([1, OUT], bf, tag="obias_row")
    nc.sync.dma_start(obias_row[:], io["obias_bf"])
    ones_row = const.tile([1, 128], bf, tag="ones_row")
    nc.sync.dma_start(ones_row[:], io["ind"][127:128, 0:128])
    gbh_sb = const.tile([128, 2], f32, tag="gbh")
    nc.sync.dma_start(gbh_sb[:], io["gbh"])
    ind_sb = const.tile([128, 256], bf, tag="ind")
    nc.sync.dma_start(ind_sb[:], io["ind"])

    def emit_tail(b, sums_bf, waA, gt):
        # ---- normalization + gating + output projection (batch tail) ----
        sums_f = gp.tile([8, Q], f32, tag="sums_f", name=f"sums_f_{b}")
        _sf = os.environ.get("SF_ENG", "dve")
        if _sf == "gps":
            nc.gpsimd.tensor_copy(sums_f[:], sums_bf[:])
        elif _sf == "act":
            nc.scalar.copy(sums_f[:], sums_bf[:])
        else:
            nc.vector.tensor_copy(sums_f[:], sums_bf[:])
        rec = gp.tile([8, Q], f32, tag="rec", name=f"rec_{b}")
        nc.vector.reciprocal_approx_fast(rec[:], sums_f[:])
        recb = gp.tile([8, Q], bf, tag="recb", name=f"recb_{b}")
        (nc.gpsimd.tensor_copy if os.environ.get("RECB_GPS", "0") == "1" else nc.vector.tensor_copy)(recb[:], rec[:])
        ga_tiles = []
        for j in range(2):
            psR = sp.tile([128, 512], f32, tag="sm", name=f"psR{j}_{b}")
            nc.tensor.matmul(
                psR[:, :Q],
                ind_sb[0:8, 128 * j : 128 * (j + 1)],
                recb[:],
                start=True,
                stop=True,
            )
            g2 = gp.tile([128, Q], bf, tag="g2", name=f"g2{j}_{b}")
            nc.vector.scalar_tensor_tensor(
                g2[:], gt[:, j, :], 1.0, psR[:, :Q], op0=ADD, op1=MUL
            )
            ga = gp.tile([128, Q], bf, tag="ga", name=f"ga{j}_{b}")
            (nc.gpsimd if os.environ.get("GA_GPS", "0") == "1" else nc.vector).tensor_tensor(
                ga[:], waA[j][:], g2[:], op=MUL
            )
            ga_tiles.append(ga)
        ob = outp.tile([128, 3, OUT], f32, tag="ob", name=f"ob_{b}")
        for qc in range(3):
            psO = sp.tile([128, 512], f32, tag="sm", name=f"psO{qc}_{b}")
            for j in range(2):
                nc.tensor.matmul(
                    psO[:, :OUT],
                    ga_tiles[j][:, 128 * qc : 128 * (qc + 1)],
                    w_sb["wo"][:, j, :],
                    start=(j == 0),
                    stop=False,
                )
            # rank-1 accumulate of o_bias: ones_col.T @ obias_row
            nc.tensor.matmul(
                psO[:, :OUT],
                ones_row[:],
                obias_row[:],
                start=False,
                stop=True,
            )
            (nc.scalar.copy if os.environ.get("OUT_ACT", "1") == "1" else nc.vector.tensor_copy)(ob[:, qc, :], psO[:, :OUT])
        (nc.scalar if os.environ.get("ODMA_ACT", "0") == "1" else nc.sync).dma_start(
            io["out"][b].rearrange("(c p) o -> p c o", p=128), ob[:]
        )

    def emit_loads_proj(b):
        # ---- loads ----
        qd = lp.tile([128, 2, Q], bf, tag="qd", name=f"qd_{b}")
        nc.sync.dma_start(qd[:], io["inT"][b, :, 0:2, :])
        md = lp.tile([128, 2, Q], bf, tag="md", name=f"md_{b}")
        nc.sync.dma_start(md[:], io["inT"][b, :, 2:4, :])
        eb = lp.tile([128, 3, Q], bf, tag="eb", name=f"eb_{b}")
        nc.sync.dma_start(eb[:], io["inT"][b, :, 4:7, :])

        # ---- projections ----
        qT = pp.tile([128, 2, Q], bf, tag="qT", name=f"qT_{b}")  # [hc, j, q]
        kT = pp.tile([128, 2, Q], bf, tag="kT", name=f"kT_{b}")  # [hc, j, k]
        gt = pp.tile([128, 2, Q], bf, tag="gt", name=f"gt_{b}")
        # [k, kc, h*33+c | 2.0]; padded to 320 so every head has a 64-wide
        # lhsT window (M=64 writes initialized junk to PSUM rows 32-63,
        # letting the pair evacuation be one full-width copy).
        vv = pp.tile([128, 3, 320], bf, tag="vv", name=f"vv_{b}")
        nc.gpsimd.memset(vv[:], 2.0)
        for j in range(2):
            ps = sp.tile([128, 512], f32, tag="sm", name=f"psq{j}_{b}")
            for a in range(2):
                nc.tensor.matmul(
                    ps[:, :Q],
                    w_sb["wq"][:, a, 128 * j : 128 * (j + 1)],
                    qd[:, a, :],
                    start=(a == 0),
                    stop=(a == 1),
                )
            (nc.scalar.copy if os.environ.get("QK_ACT", "0") in ("1", "q") else nc.vector.tensor_copy)(qT[:, j, :], ps[:, :Q])
            ps = sp.tile([128, 512], f32, tag="sm", name=f"psk{j}_{b}")
            for a in range(2):
                nc.tensor.matmul(
                    ps[:, :Q],
                    w_sb["wk"][:, a, 128 * j : 128 * (j + 1)],
                    md[:, a, :],
                    start=(a == 0),
                    stop=(a == 1),
                )
            (nc.scalar.copy if os.environ.get("QK_ACT", "0") == "1" else nc.vector.tensor_copy)(kT[:, j, :], ps[:, :Q])
            ps = sp.tile([128, 512], f32, tag="sm", name=f"psg{j}_{b}")
            for a in range(2):
                nc.tensor.matmul(
                    ps[:, :Q],
                    w_sb["wg"][:, a, 128 * j : 128 * (j + 1)],
                    qd[:, a, :],
                    start=(a == 0),
                    stop=(a == 1),
                )
            nc.scalar.activation(
                gt[:, j, :], ps[:, :Q], Tanh, bias=gbh_sb[:, j : j + 1], scale=0.5
            )
        for kc in range(3):
            ps = sp.tile([128, 512], f32, tag="sm", name=f"psv{kc}_{b}")
            for a in range(2):
                nc.tensor.matmul(
                    ps[:, :256],
                    md[:, a, 128 * kc : 128 * (kc + 1)],
                    w_sb["wv"][:, a, :],
                    start=(a == 0),
                    stop=(a == 1),
                )
            _vv_ev = nc.scalar if os.environ.get("VV_ACT", "1") == "1" else None
            if _vv_ev is not None:
                _vv_ev.copy(
                    vv[:, kc, 0:264].rearrange("p (h c) -> p h c", c=33)[:, :, 0:32],
                    ps[:, :256].rearrange("p (h c) -> p h c", c=32),
                )
            else:
                nc.vector.tensor_copy(
                    vv[:, kc, 0:264].rearrange("p (h c) -> p h c", c=33)[:, :, 0:32],
                    ps[:, :256].rearrange("p (h c) -> p h c", c=32),
                )
        # pair p's (2*sum) rows are DMA'd (engines cannot do partition-strided
        # APs) from the evacuated wa tiles into rows {2p, 2p+1} of sums_bf.
        sums_bf = gp.tile([8, Q], bf, tag="sums_bf", name=f"sums_bf_{b}")
        waA = [
            wap.tile([128, Q], bf, tag="waA", name=f"waA{j}_{b}") for j in range(2)
        ]
        return dict(
            qd=qd, md=md, eb=eb, qT=qT, kT=kT, gt=gt, vv=vv,
            sums_bf=sums_bf, waA=waA, wa_tiles=[], psW=None,
        )

    def emit_heads(b, st, heads):
        qT, kT, eb, vv = st["qT"], st["kT"], st["eb"], st["vv"]
        sums_bf, waA = st["sums_bf"], st["waA"]
        for h in heads:
            j, hh, p = h // 4, h % 4, h % 2
            psS = Sp.tile([128, 1536], f32, tag="S")
            for kc in range(3):
                nc.tensor.matmul(
                    psS[:, 512 * kc : 512 * kc + Q],
                    kT[32 * hh : 32 * (hh + 1), j, 128 * kc : 128 * (kc + 1)],
                    qT[32 * hh : 32 * (hh + 1), j, :],
                    start=True,
                    stop=True,
                    tile_position=(32 * hh, 0),
                )
            sview = psS[:].rearrange("p (c x) -> p c x", x=512)[:, :, :Q]
            if PAIR_MUL:
                if p == 0:
                    st["es2"] = wp.tile(
                        [128, 2, 3, Q], bf, tag="es2", name=f"es2_{h}_{b}"
                    )
                nc.scalar.activation(st["es2"][:, p], sview, Exp)
            else:
                es = wp.tile([128, 3, Q], bf, tag="es")
                nc.scalar.activation(es[:], sview, Exp)
                w = wp.tile([128, 3, Q], bf, tag="w")
                eng = nc.gpsimd if hh in GPS_HEADS else nc.vector
                eng.tensor_tensor(w[:], es[:], en_sb[:, h], op=MUL)
                eng.tensor_tensor(w[:], w[:], eb[:], op=MUL)
            if p == 0:
                st["psW"] = sp.tile([128, 512], f32, tag="sm", name=f"psW{h}_{b}")
            psW = st["psW"]
            if PAIR_MUL:
                if p == 1:
                    w2 = wp.tile([128, 2, 3, Q], bf, tag="w2", name=f"w2_{h}_{b}")
                    nc.vector.tensor_tensor(
                        w2[:], st["es2"][:], en_sb[:, h - 1 : h + 1], op=MUL
                    )
                    nc.vector.tensor_tensor(
                        w2[:],
                        w2[:],
                        eb[:].unsqueeze(1).broadcast_to((128, 2, 3, Q)),
                        op=MUL,
                    )
                    for hp in range(2):
                        hx = h - 1 + hp
                        for kc in range(3):
                            nc.tensor.matmul(
                                psW[64 * hp : 64 * hp + 64, :Q],
                                vv[:, kc, 33 * hx : 33 * hx + 64],
                                w2[:, hp, kc, :],
                                start=(kc == 0),
                                stop=(kc == 2),
                            )
            else:
                for kc in range(3):
                    nc.tensor.matmul(
                        psW[64 * p : 64 * p + 64, :Q],
                        vv[:, kc, 33 * h : 33 * h + 64],
                        w[:, kc, :],
                        start=(kc == 0),
                        stop=(kc == 2),
                    )
            if p == 1:
                # evacuate both heads (incl. the 2*sum rows 32 and 96)
                wa = wap.tile([128, Q], bf, tag="wa")
                _wa_mode = os.environ.get("WA_EV", "act")
                if _wa_mode == "act":
                    ev = nc.scalar
                elif _wa_mode == "dve":
                    ev = nc.vector
                else:
                    ev = nc.scalar if (h // 2) % 2 else nc.vector
                if ev is nc.scalar:
                    ev.copy(wa[:, :], psW[:, :Q])
                else:
                    ev.tensor_copy(wa[:, :], psW[:, :Q])
                pr = 2 * (h // 2)
                _dq = nc.scalar if os.environ.get("SDMA_ACT", "0") == "1" else nc.sync
                _dq.dma_start(sums_bf[pr : pr + 1, :], wa[32:33, :])
                _dq.dma_start(sums_bf[pr + 1 : pr + 2, :], wa[96:97, :])
                # rearrange both heads into the gate-aligned chunk tile
                # (engines cannot shift partition base; DMA can)
                for hx, r0 in ((h - 1, 0), (h, 64)):
                    jj, hh2 = hx // 4, hx % 4
                    _dq.dma_start(
                        waA[jj][32 * hh2 : 32 * hh2 + 32, :], wa[r0 : r0 + 32, :]
                    )
                st["wa_tiles"].append(wa)

    # Software pipeline: loads+projections of batch b, then the
    # latency-heavy tail of batch b-1 (overlapping this batch's heads).
    prev = None
    for b in range(bpc):
        st = emit_loads_proj(b)
        if prev is not None:
            emit_tail(b - 1, prev["sums_bf"], prev["waA"], prev["gt"])
        emit_heads(b, st, range(0, 8))
        prev = st
    emit_tail(bpc - 1, prev["sums_bf"], prev["waA"], prev["gt"])


def build(bpc=BPC):
    nc = bacc.Bacc(
        "TRN2",
        target_bir_lowering=False,
        debug=False,
        enable_asserts=False,
        num_devices=NUM_CORES,
    )
    f32, bf = mybir.dt.float32, mybir.dt.bfloat16
    io = {
        "inT": nc.dram_tensor("inT", [bpc, 128, 7, Q], bf, kind="ExternalInput").ap(),
        "enT": nc.dram_tensor("enT", [128, H, 3, Q], bf, kind="ExternalInput").ap(),
        "wq": nc.dram_tensor("wq", [128, 2, 256], bf, kind="ExternalInput").ap(),
        "wk": nc.dram_tensor("wk", [128, 2, 256], bf, kind="ExternalInput").ap(),
        "wv": nc.dram_tensor("wv", [128, 2, 256], bf, kind="ExternalInput").ap(),
        "wg": nc.dram_tensor("wg", [128, 2, 256], bf, kind="ExternalInput").ap(),
        "wo": nc.dram_tensor("wo", [128, 2, 256], bf, kind="ExternalInput").ap(),
        "obias_bf": nc.dram_tensor("obias_bf", [1, OUT], bf, kind="ExternalInput").ap(),
        "gbh": nc.dram_tensor("gbh", [128, 2], f32, kind="ExternalInput").ap(),
        "ind": nc.dram_tensor("ind", [128, 256], bf, kind="ExternalInput").ap(),
        "out": nc.dram_tensor("out", [bpc, Q, OUT], f32, kind="ExternalOutput").ap(),
    }
    with tile.TileContext(nc) as tc:
        with ExitStack() as ctx:
            _build_body(ctx, tc, io, bpc)
    nc.compile()
    return nc


def _prep_inputs(
    q_data,
    m_data,
    bias,
    nonbatched_bias,
    q_weights,
    k_weights,
    v_weights,
    o_weights,
    o_bias,
    gating_w,
    gating_b,
):
    """Host-side preprocessing into the DMA-friendly device layouts."""
    scale = q_weights.shape[-1] ** -0.5

    def featT(x):  # [B, S, A] -> [B, 128, A//128, S]
        b, s, a = x.shape
        t = x.transpose(0, 2, 1).reshape(b, a // 128, 128, s).transpose(0, 2, 1, 3)
        return np.ascontiguousarray(t.astype(BF16))

    qdT = featT(q_data)
    mdT = featT(m_data)
    eb = np.exp(bias[:, 0].transpose(0, 2, 1).astype(np.float32))  # [B, K, Q]
    ebT = np.ascontiguousarray(
        eb.reshape(B, 3, 128, Q).transpose(0, 2, 1, 3).astype(BF16)
    )
    en = np.exp(nonbatched_bias.transpose(0, 2, 1).astype(np.float32))  # [H, K, Q]
    enT = np.ascontiguousarray(
        en.reshape(H, 3, 128, Q).transpose(2, 0, 1, 3).astype(BF16)
    )

    def wmat(w, s=1.0):  # [A, H, hd] -> [128, 2, 256]
        m = (w.reshape(A, H * HD) * s).astype(BF16)
        return np.ascontiguousarray(m.reshape(2, 128, 256).transpose(1, 0, 2))

    wq = wmat(q_weights, scale)
    wk = wmat(k_weights)
    wv = wmat(v_weights)
    wg = wmat(gating_w)
    wo = np.ascontiguousarray(
        o_weights.reshape(256, 256).astype(BF16).reshape(2, 128, 256).transpose(1, 0, 2)
    )
    obias_bf = np.ascontiguousarray(o_bias.astype(BF16).reshape(1, OUT))
    gbh = np.ascontiguousarray(
        (0.5 * gating_b.reshape(H * HD).astype(np.float32)).reshape(2, 128).T
    )
    # indicator for the recip broadcast: row h selects the 32 output
    # partitions belonging to head h.
    ind = np.zeros((128, 256), dtype=BF16)
    for h in range(8):
        ind[h, 32 * h : 32 * (h + 1)] = 1.0
    ind[127, :] = 1.0  # ones row for the o_bias rank-1 matmul
    inT = np.ascontiguousarray(np.concatenate([qdT, mdT, ebT], axis=2))
    return dict(
        inT=inT, enT=enT, wq=wq, wk=wk, wv=wv, wg=wg, wo=wo,
        obias_bf=obias_bf, gbh=gbh, ind=ind,
    )


_NC_CACHE = {}


def kernel(**inputs):
    from concourse.bass_utils import run_bass_kernel_spmd

    full = _prep_inputs(**{k: np.asarray(v) for k, v in inputs.items()})
    if BPC not in _NC_CACHE:
        _NC_CACHE[BPC] = build(BPC)
    nc = _NC_CACHE[BPC]

    shared = {k: full[k] for k in ("enT", "wq", "wk", "wv", "wg", "wo", "obias_bf", "gbh", "ind")}
    in_maps = []
    for c in range(NUM_CORES):
        sl = slice(c * BPC, (c + 1) * BPC)
        in_maps.append(dict(inT=full["inT"][sl], **shared))

    trace = bool(int(os.environ.get("BASS_KERNEL_TRACE", "0")))
    if trace:
        try:
            from antenv.axon_hooks import get_axon_ntff_profile_hook  # noqa: F401
        except Exception:
            trace = False
    import time

    t0 = time.time()
    res = run_bass_kernel_spmd(
        nc, in_maps, core_ids=list(range(NUM_CORES)), trace=trace
    )
    kernel.last_run_wall_s = time.time() - t0
    if trace and res.exec_time_ns is not None:
        print(f"HW exec time: {res.exec_time_ns} ns")
        kernel.last_exec_time_ns = res.exec_time_ns
    out = np.concatenate([r["out"] for r in res.results], axis=0)
    return out.astype(np.float32)

